# revision 41
# baseline (speedup 1.0000x reference)
"""Multi-head attention Trainium2 Bass kernel (v2).

Shapes (hardcoded): B=4, T=2048, E=1024, H=16, DK=64.
Sharding over 8 cores: core c -> (batch b = c//2, head-group g = c%2).
Each core computes 8 heads of one batch end-to-end and a partial output
projection; the host sums the two partials per batch.

v2 design (ACT-saturation oriented; the exp stream is the roofline):
  - head-PAIR S matmuls: K/Q stored as natural f-tiles [128, T] where
    rows 0:64 = head 2i's dk and rows 64:128 = head 2i+1's dk. The S
    matmul for a key tile is a row-tiled PE pair (tile_position (0,0) /
    (64,0)) computing BOTH heads concurrently in one 512-cycle stream.
    No row duplication DMAs needed.
  - key mask applied as a per-partition bias operand of the exp
    activation (keys live on partitions of S^T): masked keys get
    bias=-30 => exp ~ 0, so they drop out of both attn@V and the
    row-sum column. V needs no masking.
  - every x chunk is loaded exactly once (f-loop inside chunk loop).
  - exp input tiles are [128, 1024] PSUM (2 banks), double buffered;
    attn@V accumulates per-head [65, 512] PSUM tiles (V plus a ones
    column producing the softmax row sums in row 64).
  - softmax normalization: row-sums bounce through DRAM to turn the
    [1, 512] sums row into [64, 16] lanes for the DVE reciprocal, then
    a stride-0 broadcast DMA replicates the reciprocals to 64
    partitions for the normalize multiply.
  - output projection per q-chunk with the bias added by the DVE
    (tensor_add with a pre-replicated bias tile) during the PSUM->SBUF
    move; no bias matmuls.
"""

import numpy as np

import concourse.bass as bass
import concourse.tile as tile
from concourse import bacc, mybir
from concourse.bass_utils import run_bass_kernel_spmd

F32 = mybir.dt.float32
BF16 = mybir.dt.bfloat16
DT = BF16
F8 = mybir.dt.float8e4
# host pre-scales the QKV projection weights by WSCALE so their
# ~N(0, 0.02) entries land in fp8e4m3's normal range; the PSUM->SBUF
# bias-add divides it back out
WSCALE = 32.0

B, T, E, H = 4, 2048, 1024, 16
DK = E // H            # 64
N_CORES = 8
FL = 512               # local f (8 heads * 64)
HL = 8                 # heads per core
HP = HL // 2           # head pairs per core = f tiles
NT = T // 128          # 16 key tiles
NE = E // 128          # 8 e tiles
NC = T // 512          # 4 chunks of 512

# Priority classes (lower = preferred by the static Tile scheduler).
# The attention stream (S-pairs, exp, attn@V, norm) keeps its natural
# emission indices (~0..20k); projection and output-projection work is
# demoted far below so it only fills engine slack and never sits ahead
# of ready attention work in an engine's static FIFO.
PRI_ATTNV = 60
PRI_NORM = 500_000
PRI_PROJ = 1_000_000
PRI_FINAL = 2_000_000


def build_nc():
    nc = bacc.Bacc("TRN2", target_bir_lowering=False, debug=False,
                   enable_asserts=False)

    qT = nc.dram_tensor("qT", [E, T], DT, kind="ExternalInput").ap()
    kT = nc.dram_tensor("kT", [E, T], DT, kind="ExternalInput").ap()
    vT = nc.dram_tensor("vT", [E, T], DT, kind="ExternalInput").ap()
    wqT = nc.dram_tensor("wqT", [E, FL], DT, kind="ExternalInput").ap()
    wkT = nc.dram_tensor("wkT", [E, FL], DT, kind="ExternalInput").ap()
    wvT = nc.dram_tensor("wvT", [E, FL], DT, kind="ExternalInput").ap()
    woT = nc.dram_tensor("woT", [FL, E], DT, kind="ExternalInput").ap()
    bqc = nc.dram_tensor("bqc", [128, HP], F32, kind="ExternalInput").ap()
    bkc = nc.dram_tensor("bkc", [128, HP], F32, kind="ExternalInput").ap()
    bvr = nc.dram_tensor("bvr", [128, FL], DT, kind="ExternalInput").ap()
    # per-key additive exp bias: 0 (allowed) or -30 (masked)
    mbd = nc.dram_tensor("mbd", [128, NT], F32, kind="ExternalInput").ap()
    out = nc.dram_tensor("out", [T, E], F32, kind="ExternalOutput").ap()

    with tile.TileContext(nc) as tc:
        with (
            tc.tile_pool(name="const", bufs=1) as constp,
            tc.tile_pool(name="qkt", bufs=1) as qktp,
            tc.tile_pool(name="vsb", bufs=1) as vsbp,
            tc.tile_pool(name="xtl", bufs=1) as xtlp,
            tc.tile_pool(name="wgt", bufs=1) as wp,
            tc.tile_pool(name="xk", bufs=4) as xkp,
            tc.tile_pool(name="xq", bufs=4) as xqp,
            tc.tile_pool(name="xv", bufs=2) as xvp,
            tc.tile_pool(name="es", bufs=6) as esp,
            tc.tile_pool(name="norm", bufs=2) as normp,
            tc.tile_pool(name="normd", bufs=2, space="DRAM") as normdp,
            tc.tile_pool(name="ob", bufs=2) as obp,
            tc.tile_pool(name="ps_s", bufs=2, space="PSUM") as ps_s,
            tc.tile_pool(name="ps_o", bufs=1, space="PSUM") as ps_o,
            tc.tile_pool(name="ps_w", bufs=2, space="PSUM") as ps_w,
        ):
            # ---- constants ----
            bq_sb = constp.tile([128, HP], F32, tag="bq")
            nc.sync.dma_start(out=bq_sb[:], in_=bqc)
            bk_sb = constp.tile([128, HP], F32, tag="bk")
            nc.sync.dma_start(out=bk_sb[:], in_=bkc)
            bv_sb = constp.tile([128, FL], DT, tag="bv")
            nc.sync.dma_start(out=bv_sb[:], in_=bvr)
            mb_sb = constp.tile([128, NT], F32, tag="mb")
            nc.sync.dma_start(out=mb_sb[:], in_=mbd)

            # persistent activations
            kt = [qktp.tile([128, T], DT, tag=f"kt{f}", name=f"kt{f}")
                  for f in range(HP)]
            qt = [qktp.tile([128, T], DT, tag=f"qt{f}", name=f"qt{f}")
                  for f in range(HP)]
            # V per key tile: [128 keys, 8 heads * 65]; per head cols
            # 0..63 = V, col 64 = ones (softmax row-sum trick)
            vt = [vsbp.tile([128, HL * 65], DT, tag=f"v{j}", name=f"v{j}")
                  for j in range(NT)]
            xtl = [xtlp.tile([128, T], DT, tag=f"x{f}", name=f"x{f}")
                   for f in range(HP)]
            for j in range(NT):
                nc.vector.memset(
                    vt[j].rearrange("p (h w) -> p h w", w=65)[:, :, 64:65],
                    1.0)

            # ---- weights (one batched DMA per tensor; DMAs issued in
            # need-order below so the first K/Q chunks aren't queued
            # behind 3.5MB of weights) ----
            # wk_sb [128, e*512 + f_cols]: e-tile e lives at cols
            # e*512..(e+1)*512
            wk_sb = wp.tile([128, NE * FL], DT, tag="wk", name="wk")
            wq_sb = wp.tile([128, NE * FL], DT, tag="wq", name="wq")
            wv_sb = wp.tile([128, NE * FL], DT, tag="wv", name="wv")
            wo_sb = wp.tile([128, HP * E], DT, tag="wo", name="wo")

            def demote(off):
                save = tc.cur_priority
                tc.cur_priority = save + off
                return save

            qT3 = qT.rearrange("(e p) t -> p e t", p=128)
            kT3 = kT.rearrange("(e p) t -> p e t", p=128)
            vT3 = vT.rearrange("(e p) t -> p e t", p=128)

            def load_chunk(pool, xdram3, c, eng=None):
                """One DMA for a [all-e, 512-q] chunk of an input."""
                xe = pool.tile([128, NE * 512], DT, tag="x", name="x")
                (eng or nc.sync).dma_start(
                    out=xe.rearrange("p (e q) -> p e q", q=512),
                    in_=xdram3[:, :, c * 512:(c + 1) * 512])
                return xe

            DR = mybir.MatmulPerfMode.DoubleRow

            def qk_proj(c, xe, w_sb, bias_sb, dst, fl, pri):
                save = demote(pri)
                for f in fl:
                    ps = ps_w.tile([128, 512], F32, tag="psw", name="psw")
                    for e in range(NE):
                        nc.tensor.matmul(
                            ps[:],
                            lhsT=w_sb[:, e * FL + f * 128:
                                      e * FL + (f + 1) * 128],
                            rhs=xe[:, e * 512:(e + 1) * 512],
                            start=(e == 0), stop=(e == NE - 1))
                    nc.vector.tensor_scalar_add(
                        dst[f][:, c * 512:(c + 1) * 512],
                        ps[:], bias_sb[:, f:f + 1])
                tc.cur_priority = save

            def v_proj(c, xe, pri):
                save = demote(pri)
                for jj in range(4):
                    j = 4 * c + jj
                    ps = ps_w.tile([128, 512], F32, tag="psw", name="psw")
                    for e in range(NE):
                        nc.tensor.matmul(
                            ps[:],
                            lhsT=xe[:, e * 512 + jj * 128:
                                    e * 512 + (jj + 1) * 128],
                            rhs=wv_sb[:, e * FL:(e + 1) * FL],
                            start=(e == 0), stop=(e == NE - 1))
                    nc.vector.tensor_add(
                        vt[j].rearrange("p (h w) -> p h w", w=65)[:, :, 0:64],
                        ps.rearrange("p (h w) -> p h w", w=64),
                        bv_sb.rearrange("p (h w) -> p h w", w=64))
                tc.cur_priority = save

            # ---- projections (all emitted up front, need-ordered).
            # K/Q f0 of chunk 0 first so the first S-pair unblocks after
            # ~16 matmuls instead of 64.
            nc.sync.dma_start(
                out=wk_sb.rearrange("p (e f) -> p e f", f=FL),
                in_=wkT.rearrange("(e p) f -> p e f", p=128))
            xk0 = load_chunk(xkp, kT3, 0)
            nc.sync.dma_start(
                out=wq_sb.rearrange("p (e f) -> p e f", f=FL),
                in_=wqT.rearrange("(e p) f -> p e f", p=128))
            xq0 = load_chunk(xqp, qT3, 0)
            # Units run HP-MAJOR (hp0 over all 4 q-chunks first), so the
            # first 64 exps need only the f0 K/Q projections plus V --
            # projection demand is spread evenly over the whole run.
            # Emission/need order: K f0 + V interleaved (attn@V of hp0
            # trails its exps by a few ktiles), then Q f0 per q-chunk,
            # then the f1..f3 passes.
            qk_proj(0, xk0, wk_sb, bk_sb, kt, [0], PRI_PROJ)
            qk_proj(0, xq0, wq_sb, bq_sb, qt, [0], PRI_PROJ)
            nc.sync.dma_start(
                out=wv_sb.rearrange("p (e f) -> p e f", f=FL),
                in_=wvT.rearrange("(e p) f -> p e f", p=128))
            xv0 = load_chunk(xvp, vT3, 0)
            v_proj(0, xv0, PRI_PROJ)
            xk1 = load_chunk(xkp, kT3, 1)
            qk_proj(1, xk1, wk_sb, bk_sb, kt, [0], PRI_PROJ)
            xv1 = load_chunk(xvp, vT3, 1)
            v_proj(1, xv1, PRI_PROJ)
            xk2 = load_chunk(xkp, kT3, 2)
            qk_proj(2, xk2, wk_sb, bk_sb, kt, [0], PRI_PROJ)
            xv2 = load_chunk(xvp, vT3, 2)
            v_proj(2, xv2, PRI_PROJ)
            xk3 = load_chunk(xkp, kT3, 3)
            qk_proj(3, xk3, wk_sb, bk_sb, kt, [0], PRI_PROJ)
            xv3 = load_chunk(xvp, vT3, 3)
            v_proj(3, xv3, PRI_PROJ)
            xks = [xk0, xk1, xk2, xk3]
            xqs = [xq0]
            for c in range(1, NC):
                xq = load_chunk(xqp, qT3, c)
                xqs.append(xq)
                qk_proj(c, xq, wq_sb, bq_sb, qt, [0], PRI_PROJ)
            nc.sync.dma_start(
                out=wo_sb.rearrange("p (f e) -> p f e", e=E),
                in_=woT.rearrange("(f p) e -> p f e", p=128))
            for f in range(1, HP):
                for c in range(NC):
                    qk_proj(c, xks[c], wk_sb, bk_sb, kt, [f], PRI_PROJ)
                for c in range(NC):
                    qk_proj(c, xqs[c], wq_sb, bq_sb, qt, [f], PRI_PROJ)

            # ---- attention + output projection ----
            def unit(hp, qc, nrm_eng=None):
                """One (head-pair, q-chunk-512) attention unit."""
                qsl = slice(qc * 512, (qc + 1) * 512)
                psoA = ps_o.tile([65, 512], F32, tag="psoA", name="psoA")
                psoB = ps_o.tile([65, 512], F32, tag="psoB", name="psoB")
                h0 = 2 * hp
                for k in range(NT):
                    pss = ps_s.tile([128, 1024], F32, tag="pss",
                                    name="pss")
                    for t in range(2):
                        r = slice(t * 64, (t + 1) * 64)
                        nc.tensor.matmul(
                            pss[:, t * 512:(t + 1) * 512],
                            lhsT=kt[hp][r, k * 128:(k + 1) * 128],
                            rhs=qt[hp][r, qsl],
                            start=True, stop=True,
                            tile_position=(t * 64, 0))
                    es = esp.tile([128, 1024], DT, tag="es", name="es")
                    nc.scalar.activation(
                        out=es[:], in_=pss[:],
                        func=mybir.ActivationFunctionType.Exp,
                        bias=mb_sb[:, k:k + 1], scale=0.125)
                    # small demotion: the tail attn@V of a unit must not
                    # sit ahead of the next unit's S-pairs in the PE FIFO
                    sv = demote(PRI_ATTNV)
                    nc.tensor.matmul(
                        psoA[:], lhsT=vt[k][:, h0 * 65:(h0 + 1) * 65],
                        rhs=es[:, 0:512],
                        start=(k == 0), stop=(k == NT - 1))
                    nc.tensor.matmul(
                        psoB[:], lhsT=vt[k][:, (h0 + 1) * 65:(h0 + 2) * 65],
                        rhs=es[:, 512:1024],
                        start=(k == 0), stop=(k == NT - 1))
                    tc.cur_priority = sv
                # Copy the accumulators to SBUF right away so the single
                # pso PSUM buffer is freed for the next unit; the whole
                # normalization chain then runs from SBUF off the PSUM
                # critical path. Rows 0..63 = O^T, row 64 = sum(exp).
                ot = normp.tile([65, 1024], F32, tag="ot", name="ot")
                nc.vector.tensor_copy(out=ot[:, 0:512], in_=psoA[:])
                nc.vector.tensor_copy(out=ot[:, 512:1024], in_=psoB[:])
                # The 4-hop DRAM bounce (repartition sums row -> 64-lane
                # reciprocal -> partition broadcast) runs on the
                # otherwise-idle GpSimd SWDGE queue and at demoted
                # priority: it gates only the output projection, and it
                # must never block the Sync DMA queue or the DVE ahead
                # of the next unit's pso-freeing copies.
                save = demote(PRI_NORM)
                nrm = nrm_eng or nc.gpsimd
                rsd = normdp.tile([1, 1024], F32, tag="rsd", name="rsd")
                nrm.dma_start(out=rsd[:], in_=ot[64:65, :])
                rs = normp.tile([64, 16], F32, tag="rs", name="rs")
                nrm.dma_start(
                    out=rs[:],
                    in_=rsd.rearrange("o (p w) -> (o p) w", w=16))
                ri = normp.tile([64, 16], DT, tag="ri", name="ri")
                with nc.allow_low_precision(
                        reason="softmax denominators are O(1e3); bf16 "
                               "reciprocal adds ~0.4% matching the bf16 "
                               "xtl quantization already present"):
                    nc.vector.reciprocal(ri[:], rs[:])
                rid = normdp.tile([64, 16], DT, tag="rid", name="rid")
                nrm.dma_start(out=rid[:], in_=ri[:])
                rep = normp.tile([64, 1024], DT, tag="rep", name="rep")
                nrm.dma_start(
                    out=rep[:],
                    in_=rid.rearrange("p w -> () (p w)").to_broadcast(
                        [64, 1024]))
                nc.vector.tensor_mul(
                    xtl[hp][0:64, qsl], ot[0:64, 0:512], rep[:, 0:512])
                nc.vector.tensor_mul(
                    xtl[hp][64:128, qsl], ot[0:64, 512:1024],
                    rep[:, 512:1024])
                tc.cur_priority = save

            def final_group(j, c2, on_act=False):
                """Output projection for q rows j*128.. and E half c2.
                b_o is added host-side after the partial-sum gather, so
                the PSUM->SBUF move is a plain copy; the tail groups run
                it on the by-then-idle scalar engine instead of the DVE.
                """
                save = demote(PRI_FINAL)
                ps = ps_w.tile([128, 512], F32, tag="psw", name="psf")
                for f in range(HP):
                    nc.tensor.matmul(
                        ps[:],
                        lhsT=xtl[f][:, j * 128:(j + 1) * 128],
                        rhs=wo_sb[:, f * E + c2 * 512:
                                  f * E + (c2 + 1) * 512],
                        start=(f == 0), stop=(f == HP - 1))
                ob = obp.tile([128, 512], F32, tag="ob", name="ob")
                if on_act:
                    nc.scalar.copy(ob[:], ps[:])
                else:
                    nc.vector.tensor_copy(ob[:], ps[:])
                nc.sync.dma_start(
                    out=out[j * 128:(j + 1) * 128,
                            c2 * 512:(c2 + 1) * 512],
                    in_=ob[:])
                tc.cur_priority = save

            # Units run hp-major; each qc's output projection becomes
            # ready after hp3's unit for that qc and is emitted right
            # after it (demoted, so it fills engine slack).
            for hp in range(HP):
                for qc in range(NC):
                    last = (hp == HP - 1 and qc == NC - 1)
                    unit(hp, qc, nc.sync if last else None)
                    if hp == HP - 1:
                        for j in range(4 * qc, 4 * qc + 4):
                            for c2 in range(2):
                                final_group(j, c2, on_act=last)

    nc.compile()
    return nc


_NC_CACHE = None


def _get_nc():
    global _NC_CACHE
    if _NC_CACHE is None:
        _NC_CACHE = build_nc()
    return _NC_CACHE


def make_in_maps(query, key_, value, mask, w_q, b_q, w_k, b_k, w_v, b_v,
                 w_o, b_o):
    import ml_dtypes
    f32 = np.float32
    bf16 = ml_dtypes.bfloat16
    fp8 = ml_dtypes.float8_e4m3
    c = lambda a: np.ascontiguousarray(a).astype(bf16)
    c8 = lambda a: np.ascontiguousarray(a).astype(fp8)
    in_maps = []
    for core in range(N_CORES):
        b, g = core // 2, core % 2
        fs = slice(g * FL, (g + 1) * FL)
        mb = np.where(np.asarray(mask[b]).reshape(NT, 128).T,
                      0.0, -30.0).astype(f32)
        in_maps.append({
            "qT": c(query[b].T.astype(f32, copy=False)),
            "kT": c(key_[b].T.astype(f32, copy=False)),
            "vT": c(value[b].T.astype(f32, copy=False)),
            "wqT": c(w_q[fs, :].T.astype(f32, copy=False)),
            "wkT": c(w_k[fs, :].T.astype(f32, copy=False)),
            "wvT": c(w_v[fs, :].T.astype(f32, copy=False)),
            "woT": c(w_o[:, fs].T.astype(f32, copy=False)),
            "bqc": np.ascontiguousarray(
                b_q[fs].astype(f32, copy=False).reshape(HP, 128).T),
            "bkc": np.ascontiguousarray(
                b_k[fs].astype(f32, copy=False).reshape(HP, 128).T),
            "bvr": c(np.broadcast_to(b_v[fs], (128, FL))),
            "mbd": np.ascontiguousarray(mb),
        })
    return in_maps


def kernel(query=None, key_=None, value=None, mask=None, w_q=None, b_q=None,
           w_k=None, b_k=None, w_v=None, b_v=None, w_o=None, b_o=None,
           key=None, **_kwargs):
    if key_ is None:
        key_ = key
    args = [np.asarray(a) for a in
            (query, key_, value, mask, w_q, b_q, w_k, b_k, w_v, b_v,
             w_o, b_o)]
    nc = _get_nc()
    in_maps = make_in_maps(*args)
    res = run_bass_kernel_spmd(nc, in_maps, core_ids=list(range(N_CORES)))
    outs = [res.results[i]["out"] for i in range(N_CORES)]
    bo_row = np.asarray(b_o, np.float32).reshape(1, E)
    full = np.empty((B, T, E), np.float32)
    for b in range(B):
        full[b] = outs[2 * b] + outs[2 * b + 1] + bo_row
    return full
